# revision 24
# baseline (speedup 1.0000x reference)
"""GAT/GRAN message-passing kernel for 8 Trainium2 NeuronCores.

Strategy:
  - Edges partitioned by dst-node range: core c owns dst rows
    [c*6250, (c+1)*6250); scatter-add and GRU for those rows are local.
  - The execution backend's cost is dominated by STATIC program size
    (~50-100us per instruction; dynamic iterations of hardware loops are
    nearly free), so the per-window edge phase and the GRU phase run as
    For_i hardware loops: ~250 static instructions total instead of ~5k.
  - Host->device bytes are the other dominant cost, so the node table is
    shipped SHARDED (each core gets only its 1.6MB slab, bf16) and
    all-gathered on device into a DRAM table for the src gathers; gather
    index tensors ship in the unreplicated 16-row wrap layout and are
    replicated to 128 partitions on device; iota/identity constants are
    tiny or built on device; output returns transposed bf16.
  - Node-state gathers use the gpsimd dma_gather custom instruction
    (transposed mode, bf16), landing features-on-partitions for the edge
    MLP matmuls. int16 indices address the padded global table (row =
    core*6272 + local), split into overlapping lo/hi views of the
    all-gathered table to cover 50176 > 32768 rows.
  - Edge MLP layer 1 uses linearity: W1^T(xs-xd) = W1^T xs + (-W1)^T xd,
    accumulated in PSUM; per-window aggregation is a one-hot matmul into
    a PSUM tile; GRU runs transposed (features on partitions) and writes
    the output transposed, un-transposed on host.
"""

import math
import sys
from dataclasses import dataclass

import numpy as np

sys.path.insert(0, "/opt/trn_rl_repo")

from contextlib import ExitStack

from concourse import bacc, bass, mybir, tile  # noqa: E402
from concourse.bass import ds, ts  # noqa: E402

F32 = mybir.dt.float32
BF16 = mybir.dt.bfloat16
FP8 = mybir.dt.float8e4
I16 = mybir.dt.int16
NP_FP8 = mybir.dt.np(mybir.dt.float8e4)
EF_FP8 = True  # ship edge features as fp8-e4m3 (halves the biggest transfer)
AF = mybir.ActivationFunctionType
OP = mybir.AluOpType
NP_BF16 = mybir.dt.np(BF16)

D = 128  # node state dim == msg dim
E = 32   # edge attr dim
WIN = 128  # nodes per aggregation window
MBX = 4    # 128-edge blocks per macro tile
LO = 32768  # dma_gather int16 index limit


@dataclass
class Geom:
    N: int = 50000
    M: int = 800000
    NCORES: int = 8

    @property
    def NPC(self):  # nodes per core
        return self.N // self.NCORES

    @property
    def NWIN(self):
        return math.ceil(self.NPC / WIN)

    @property
    def NPAD(self):
        return self.NWIN * WIN

    @property
    def TOT(self):  # padded global table rows
        return self.NCORES * self.NPAD

    @property
    def LO_ROWS(self):
        return min(self.TOT, LO)

    @property
    def HIB(self):  # hi view base row
        return max(self.TOT - LO, 0)

    @property
    def HI_ROWS(self):
        return self.TOT - self.HIB


def build_program(g: Geom, NB: int, TA: int, reps: int = 1):
    """Build the SPMD per-core program. NB = 128-edge blocks per window;
    blocks [0,TA) gather src from the lo table view, the rest from hi."""
    NMT = math.ceil(NB / MBX)
    LW = NB * 8          # idx columns per window (wrap16)
    SL = NB * 128        # edge slots per window
    nc = bacc.Bacc(
        "TRN2", target_bir_lowering=False, debug=False, num_devices=g.NCORES
    )

    slab = nc.dram_tensor("slab", [g.NPAD, D], BF16, kind="ExternalInput").ap()
    sidx16 = nc.dram_tensor("sidx16", [16, g.NWIN * LW], I16, kind="ExternalInput").ap()
    didx16 = nc.dram_tensor("didx16", [16, g.NWIN * LW], I16, kind="ExternalInput").ap()
    dlocD = nc.dram_tensor("dlocD", [128, g.NWIN * NB], BF16, kind="ExternalInput").ap()
    efTD = nc.dram_tensor(
        "efTD", [E, g.NWIN * SL], FP8 if EF_FP8 else BF16, kind="ExternalInput"
    ).ap()
    wmat = nc.dram_tensor("wmat", [6 * 128, D], BF16, kind="ExternalInput").ap()
    wgru = nc.dram_tensor("wgru", [128, 768], BF16, kind="ExternalInput").ap()
    bias = nc.dram_tensor("bias", [128, 8], F32, kind="ExternalInput").ap()
    iotaD = nc.dram_tensor("iotaD", [128, 128], BF16, kind="ExternalInput").ap()
    outT = nc.dram_tensor("outT", [128, g.NPAD], BF16, kind="ExternalOutput").ap()
    nbounce = nc.dram_tensor("nbounce", [g.NPAD, D], BF16).ap()
    ntab = nc.dram_tensor("ntab", [g.TOT, D], BF16).ap()
    ntab_lo = ntab[0:g.LO_ROWS]
    ntab_hi = ntab[g.HIB:g.TOT]

    with tile.TileContext(nc) as tc, ExitStack() as ctx:
        cpool = ctx.enter_context(tc.tile_pool(name="const", bufs=1))
        epool = ctx.enter_context(tc.tile_pool(name="edge", bufs=2))
        gpool = ctx.enter_context(tc.tile_pool(name="gru", bufs=2))
        ppool = ctx.enter_context(tc.tile_pool(name="pwork", bufs=5, space="PSUM"))
        apool = ctx.enter_context(tc.tile_pool(name="pagg", bufs=1, space="PSUM"))

        # ---- constants -------------------------------------------------
        wm = cpool.tile([128, 6, D], BF16)
        nc.sync.dma_start(wm[:], wmat.rearrange("(k p) d -> p k d", p=128))
        bs = cpool.tile([128, 8], F32)
        nc.sync.dma_start(bs[:], bias[:, :])
        wg = cpool.tile([128, 768], BF16)
        nc.sync.dma_start(wg[:], wgru[:, :])
        ion = cpool.tile([128, 128], BF16)
        nc.sync.dma_start(ion[:], iotaD[:, :])

        # gather indices: load 16-row wrapped form, replicate to 128 parts
        six = cpool.tile([128, g.NWIN * LW], I16)
        nc.sync.dma_start(six[0:16, :], sidx16[:, :])
        dix = cpool.tile([128, g.NWIN * LW], I16)
        nc.sync.dma_start(dix[0:16, :], didx16[:, :])
        for p in (16, 32, 64):
            nc.sync.dma_start(six[p:2 * p, :], six[0:p, :])
            nc.sync.dma_start(dix[p:2 * p, :], dix[0:p, :])
        dlc = cpool.tile([128, g.NWIN * NB], BF16)
        nc.sync.dma_start(dlc[:], dlocD[:, :])

        # node table: all-gather the slabs into DRAM
        nc.sync.dma_start(nbounce[:, :], slab[:, :])
        if g.NCORES > 1:
            nc.gpsimd.collective_compute(
                "AllGather",
                mybir.AluOpType.bypass,
                replica_groups=[list(range(g.NCORES))],
                ins=[nbounce[:, :].opt()],
                outs=[ntab[:, :].opt()],
            )
        else:
            nc.sync.dma_start(ntab[:, :], nbounce[:, :])

        # xTb = transposed local slab (features on partitions), bf16
        xTb = cpool.tile([128, g.NPAD], BF16)
        nc.sync.dma_start_transpose(xTb[:], slab[:, :])

        # aggregated-message staging (transposed, bf16)
        stg = cpool.tile([128, g.NPAD], BF16)

        W1d, A1d = wm[:, 0, :], wm[:, 1, :]
        W2, A2 = wm[:, 2, :], wm[:, 3, :]
        W1e, A1e = wm[:32, 4, :], wm[:32, 5, :]
        Wi_r, Wi_z, Wi_n = wg[:, 0:128], wg[:, 128:256], wg[:, 256:384]
        Wh_r, Wh_z, Wh_n = wg[:, 384:512], wg[:, 512:640], wg[:, 640:768]

        for _rep in range(reps):
            # ---- edge phase (hardware loop over windows) ---------------
            with tc.For_i(0, g.NWIN, 1) as w:
                if EF_FP8:
                    ef8 = epool.tile([E, SL], FP8, tag="ef8")
                    nc.sync.dma_start(ef8[:], efTD[:, ts(w, SL)])
                    ef = epool.tile([E, SL], BF16, tag="ef")
                    nc.vector.tensor_copy(ef[:], ef8[:])
                else:
                    ef = epool.tile([E, SL], BF16, tag="ef")
                    nc.sync.dma_start(ef[:], efTD[:, ts(w, SL)])
                dl = epool.tile([128, NB], BF16, tag="dl")
                nc.vector.tensor_copy(dl[:], dlc[:, ts(w, NB)])
                S = epool.tile([128, SL], BF16, tag="S")
                nc.vector.tensor_tensor(
                    S[:].rearrange("p (b j) -> p b j", b=NB),
                    dl[:].to_broadcast([128, NB, 128]),
                    ion[:].rearrange("p (o j) -> p o j", o=1).to_broadcast(
                        [128, NB, 128]
                    ),
                    op=OP.is_equal,
                )

                def gather_region(out_tile, tab, idx_tile, idx_off, nidx):
                    done = 0
                    while done < nidx:
                        n = min(512, nidx - done)
                        o0 = idx_off + done
                        nc.gpsimd.dma_gather(
                            out_ap=out_tile[:, o0:o0 + n].rearrange(
                                "p (o x) -> p o x", o=1
                            ),
                            in_ap=tab,
                            idxs_ap=idx_tile[:, ds(w * LW + o0 // 16, n // 16)],
                            num_idxs=n,
                            num_idxs_reg=n,
                            elem_size=D,
                            transpose=True,
                        )
                        done += n

                xs = epool.tile([128, SL], BF16, tag="xs")
                gather_region(xs, ntab_lo, six, 0, TA * 128)
                gather_region(xs, ntab_hi, six, TA * 128, (NB - TA) * 128)
                xd = epool.tile([128, SL], BF16, tag="xd")
                gather_region(xd, slab, dix, 0, NB * 128)
                diff = epool.tile([128, SL], BF16, tag="diff")
                nc.vector.tensor_sub(diff[:], xs[:], xd[:])

                agg = apool.tile([128, WIN], F32, space="PSUM", tag="agg")
                for t in range(NMT):
                    mb = min(MBX, NB - t * MBX)
                    width = mb * 128
                    sl_ = slice(t * MBX * 128, t * MBX * 128 + width)
                    xst, eft = diff[:, sl_], ef[:, sl_]
                    halves = [
                        slice(h * 512, min((h + 1) * 512, width))
                        for h in range(math.ceil(width / 512))
                    ]
                    h1 = ppool.tile([128, width], F32, space="PSUM", tag="ps")
                    a1 = ppool.tile([128, width], F32, space="PSUM", tag="ps")
                    for hs in halves:
                        nc.tensor.matmul(h1[:, hs], W1d, xst[:, hs], start=True, stop=False)
                        nc.tensor.matmul(h1[:, hs], W1e, eft[:, hs], start=False, stop=True)
                        nc.tensor.matmul(a1[:, hs], A1d, xst[:, hs], start=True, stop=False)
                        nc.tensor.matmul(a1[:, hs], A1e, eft[:, hs], start=False, stop=True)
                    h1r = epool.tile([128, width], BF16, tag="h1r")
                    nc.scalar.activation(h1r[:], h1[:], AF.Relu, bias=bs[:, 0:1])
                    a1r = epool.tile([128, width], BF16, tag="a1r")
                    nc.scalar.activation(a1r[:], a1[:], AF.Relu, bias=bs[:, 1:2])

                    msgT = ppool.tile([128, width], F32, space="PSUM", tag="ps")
                    attT = ppool.tile([128, width], F32, space="PSUM", tag="ps")
                    for hs in halves:
                        nc.tensor.matmul(msgT[:, hs], W2, h1r[:, hs], start=True, stop=True)
                        nc.tensor.matmul(attT[:, hs], A2, a1r[:, hs], start=True, stop=True)
                    atts = epool.tile([128, width], BF16, tag="atts")
                    nc.scalar.activation(atts[:], attT[:], AF.Sigmoid, bias=bs[:, 3:4])
                    gT = epool.tile([128, width], BF16, tag="gT")
                    nc.vector.scalar_tensor_tensor(
                        gT[:], msgT[:], bs[:, 2:3], atts[:], op0=OP.add, op1=OP.mult
                    )

                    gs = epool.tile([128, width], BF16, tag="gs")
                    nc.sync.dma_start_transpose(
                        gs[:].rearrange("p (b f) -> p b f", b=mb), gT[:]
                    )
                    for b in range(mb):
                        blk = t * MBX + b
                        nc.tensor.matmul(
                            agg[:],
                            gs[:, b * 128:(b + 1) * 128],
                            S[:, blk * 128:(blk + 1) * 128],
                            start=(blk == 0),
                            stop=(blk == NB - 1),
                            skip_group_check=True,
                        )
                nc.vector.tensor_copy(stg[:, ts(w, WIN)], agg[:])

            # ---- GRU phase (hardware loop over 512-node chunks) --------
            def gru_chunk(cslice, cw):
                ag = stg[:, cslice]
                hT = xTb[:, cslice]
                rp = ppool.tile([128, cw], F32, space="PSUM", tag="ps")
                nc.tensor.matmul(rp[:], Wi_r, ag, start=True, stop=False)
                nc.tensor.matmul(rp[:], Wh_r, hT, start=False, stop=True)
                rT = gpool.tile([128, cw], F32, tag="rT")
                nc.scalar.activation(rT[:], rp[:], AF.Sigmoid, bias=bs[:, 4:5])
                zp = ppool.tile([128, cw], F32, space="PSUM", tag="ps")
                nc.tensor.matmul(zp[:], Wi_z, ag, start=True, stop=False)
                nc.tensor.matmul(zp[:], Wh_z, hT, start=False, stop=True)
                zT = gpool.tile([128, cw], F32, tag="zT")
                nc.scalar.activation(zT[:], zp[:], AF.Sigmoid, bias=bs[:, 5:6])
                gin = ppool.tile([128, cw], F32, space="PSUM", tag="ps")
                nc.tensor.matmul(gin[:], Wi_n, ag, start=True, stop=True)
                ghn = ppool.tile([128, cw], F32, space="PSUM", tag="ps")
                nc.tensor.matmul(ghn[:], Wh_n, hT, start=True, stop=True)
                rg = gpool.tile([128, cw], F32, tag="rg")
                nc.vector.scalar_tensor_tensor(
                    rg[:], ghn[:], bs[:, 7:8], rT[:], op0=OP.add, op1=OP.mult
                )
                npre = gpool.tile([128, cw], F32, tag="npre")
                nc.vector.tensor_add(npre[:], rg[:], gin[:])
                nT = gpool.tile([128, cw], F32, tag="nT")
                nc.scalar.activation(nT[:], npre[:], AF.Tanh, bias=bs[:, 6:7])
                hTf = gpool.tile([128, cw], F32, tag="hTf")
                nc.vector.tensor_copy(hTf[:], hT)
                hmn = gpool.tile([128, cw], F32, tag="hmn")
                nc.vector.tensor_sub(hmn[:], hTf[:], nT[:])
                zh = gpool.tile([128, cw], F32, tag="zh")
                nc.vector.tensor_mul(zh[:], zT[:], hmn[:])
                nw = gpool.tile([128, cw], BF16, tag="nw")
                nc.vector.tensor_add(nw[:], nT[:], zh[:])
                nc.sync.dma_start(outT[:, cslice], nw[:])

            nfull = g.NPAD // 512
            if nfull > 0:
                with tc.For_i(0, nfull, 1) as c:
                    gru_chunk(ts(c, 512), 512)
            tail = g.NPAD - nfull * 512
            if tail:
                gru_chunk(slice(nfull * 512, g.NPAD), tail)

    nc.compile()
    return nc


_TPOOL = None


def _tpool():
    global _TPOOL
    if _TPOOL is None:
        from concurrent.futures import ThreadPoolExecutor

        _TPOOL = ThreadPoolExecutor(8)
    return _TPOOL


def _par_cast(src, np_dt, nchunk=8):
    """Parallel elementwise dtype cast of a 2D array (rows chunked)."""
    out = np.empty(src.shape, np_dt)
    rows = src.shape[0]
    step = (rows + nchunk - 1) // nchunk

    def do(i):
        s = slice(i * step, min((i + 1) * step, rows))
        out[s] = src[s]

    list(_tpool().map(do, range(nchunk)))
    return out


def prep_counts(g: Geom, inputs: dict):
    """Phase A: group edges by (dst-core, dst-window, lo/hi-src) and size
    the uniform block padding. Cheap; determines program shape (NB, TA)."""
    ei = np.asarray(inputs["edge_index"])
    src = ei[0].astype(np.int64)
    dst = ei[1].astype(np.int64)
    NPC, NWIN, NPAD = g.NPC, g.NWIN, g.NPAD
    core = dst // NPC
    rem = dst - core * NPC
    gwin = core * NWIN + (rem >> 7)
    srow = (src // NPC) * NPAD + (src % NPC)  # padded global row
    isB = srow >= g.LO_ROWS
    grp = (gwin << 1) | isB

    order = np.argsort(grp, kind="stable")
    cnt = np.bincount(grp, minlength=g.NCORES * NWIN * 2)
    cntA, cntB = cnt[0::2], cnt[1::2]
    TA = int(math.ceil(cntA.max() / 128.0)) if cntA.max() else 0
    TB = int(math.ceil(cntB.max() / 128.0)) if cntB.max() else 0
    NB = max(TA + TB, 1)

    starts = np.concatenate([[0], np.cumsum(cnt)])[:-1]
    rank = np.empty(len(src), np.int64)
    rank[order] = np.arange(len(src)) - starts[grp[order]]
    slot = np.where(isB, TA * 128 + rank, rank)  # slot per ORIGINAL edge
    ci = core
    wi = gwin - core * NWIN
    state = dict(ci=ci, wi=wi, slot=slot, srow=srow, rem=rem, isB=isB)
    return state, NB, TA


def prep_arrays(g: Geom, inputs: dict, state, NB: int, TA: int):
    """Phase B: yield (name, global_array) in upload order — small/cheap
    tensors first so their transfers overlap the edge-feature build."""
    NPC, NWIN, NPAD = g.NPC, g.NWIN, g.NPAD
    SL = NB * 128
    ci, wi, slot = state["ci"], state["wi"], state["slot"]

    def wrap16(a):  # [C, NWIN, SL] -> [C*16, NWIN*NB*8]
        return np.ascontiguousarray(
            a.reshape(g.NCORES, NWIN, SL // 16, 16)
            .transpose(0, 3, 1, 2)
            .reshape(g.NCORES * 16, NWIN * (SL // 16))
        )

    for name, arr in _weight_arrays(g, inputs):
        yield name, arr

    nf = np.asarray(inputs["node_feat"], np.float32)
    slab_g = np.zeros((g.NCORES, NPAD, D), NP_BF16)
    slab_g[:, :NPC] = nf.reshape(g.NCORES, NPC, D).astype(NP_BF16)
    yield "slab", slab_g.reshape(g.NCORES * NPAD, D)

    sidxpad = np.zeros((g.NCORES, NWIN, SL), np.int16)
    sidxpad[ci, wi, slot] = np.where(
        state["isB"], state["srow"] - g.HIB, state["srow"]
    ).astype(np.int16)
    yield "sidx16", wrap16(sidxpad)
    didxpad = np.zeros((g.NCORES, NWIN, SL), np.int16)
    didxpad[ci, wi, slot] = state["rem"].astype(np.int16)
    yield "didx16", wrap16(didxpad)
    dlocpad = np.full((g.NCORES, NWIN, SL), -1.0, NP_BF16)
    dlocpad[ci, wi, slot] = (state["rem"] & 127).astype(NP_BF16)
    yield "dlocD", np.ascontiguousarray(
        dlocpad.reshape(g.NCORES, NWIN, NB, 128)
        .transpose(0, 3, 1, 2)
        .reshape(g.NCORES * 128, NWIN * NB)
    )

    np_ef = NP_FP8 if EF_FP8 else NP_BF16
    efq = _par_cast(np.asarray(inputs["edge_feat"], np.float32), np_ef)
    efpad = np.zeros((g.NCORES, NWIN, SL, E), np_ef)
    efpad[ci, wi, slot] = efq
    yield "efTD", np.ascontiguousarray(
        efpad.reshape(g.NCORES, NWIN * SL, E)
        .transpose(0, 2, 1)
        .reshape(g.NCORES * E, NWIN * SL)
    )


def prep_inputs(g: Geom, inputs: dict):
    state, NB, TA = prep_counts(g, inputs)
    gmaps = dict(prep_arrays(g, inputs, state, NB, TA))
    return gmaps, NB, TA


def _weight_arrays(g: Geom, inputs: dict):
    msg_W1 = np.asarray(inputs["msg_W1"], np.float32)
    att_W1 = np.asarray(inputs["att_W1"], np.float32)
    wmat = np.zeros((6, 128, D), np.float32)
    wmat[0] = msg_W1[:128]
    wmat[1] = att_W1[:128]
    wmat[2] = np.asarray(inputs["msg_W2"], np.float32)
    wmat[3] = np.asarray(inputs["att_W2"], np.float32)
    wmat[4, :32] = msg_W1[128:160]
    wmat[5, :32] = att_W1[128:160]
    wmat_b = wmat.reshape(6 * 128, D).astype(NP_BF16)
    yield "wmat", np.ascontiguousarray(
        np.broadcast_to(wmat_b, (g.NCORES, 6 * 128, D)).reshape(-1, D)
    )
    wgru = np.concatenate(
        [np.asarray(inputs["gru_Wi"], np.float32),
         np.asarray(inputs["gru_Wh"], np.float32)], axis=1
    ).astype(NP_BF16)
    yield "wgru", np.ascontiguousarray(
        np.broadcast_to(wgru, (g.NCORES, 128, 768)).reshape(-1, 768)
    )
    bi = np.asarray(inputs["gru_bi"], np.float32)
    bh = np.asarray(inputs["gru_bh"], np.float32)
    bias = np.stack(
        [
            np.asarray(inputs["msg_b1"], np.float32),
            np.asarray(inputs["att_b1"], np.float32),
            np.asarray(inputs["msg_b2"], np.float32),
            np.asarray(inputs["att_b2"], np.float32),
            (bi + bh)[0:128],
            (bi + bh)[128:256],
            bi[256:384],
            bh[256:384],
        ],
        axis=1,
    )
    yield "bias", np.ascontiguousarray(
        np.broadcast_to(bias, (g.NCORES, 128, 8)).reshape(-1, 8)
    )
    iota = np.broadcast_to(
        np.arange(128, dtype=np.float32), (128, 128)
    ).astype(NP_BF16)
    yield "iotaD", np.ascontiguousarray(
        np.broadcast_to(iota, (g.NCORES, 128, 128)).reshape(-1, 128)
    )


# ---------------------------------------------------------------------------
# Cached PJRT runner (avoids per-call retrace / zero-buffer reupload)
# ---------------------------------------------------------------------------

_RUNNERS = {}


def _get_runner(nc, n_cores: int):
    key = id(nc)
    if key in _RUNNERS:
        return _RUNNERS[key]
    import jax
    from jax.experimental.shard_map import shard_map
    from jax.sharding import Mesh, NamedSharding, PartitionSpec

    from concourse import bass2jax
    from concourse import mybir as mb

    bass2jax.install_neuronx_cc_hook()

    partition_name = nc.partition_id_tensor.name if nc.partition_id_tensor else None
    in_names, out_names, out_avals, zero_shapes = [], [], [], []
    for alloc in nc.m.functions[0].allocations:
        if not isinstance(alloc, mb.MemoryLocationSet):
            continue
        name = alloc.memorylocations[0].name
        if alloc.kind == "ExternalInput":
            if name != partition_name:
                in_names.append(name)
        elif alloc.kind == "ExternalOutput":
            out_names.append(name)
            shape = tuple(alloc.tensor_shape)
            dtype = mb.dt.np(alloc.dtype)
            out_avals.append(jax.core.ShapedArray(shape, dtype))
            zero_shapes.append((shape, dtype))
    n_params = len(in_names)
    all_names = list(in_names) + list(out_names)
    if partition_name is not None:
        all_names.append(partition_name)

    assert nc.dbg_addr is None, "build with debug=False"

    def _body(*args):
        operands = list(args)
        if partition_name is not None:
            operands.append(bass2jax.partition_id_tensor())
        outs = bass2jax._bass_exec_p.bind(
            *operands,
            out_avals=tuple(out_avals),
            in_names=tuple(all_names),
            out_names=tuple(out_names),
            lowering_input_output_aliases=(),
            sim_require_finite=True,
            sim_require_nnan=True,
            nc=nc,
        )
        return tuple(outs)

    devices = jax.devices()[:n_cores]
    mesh = Mesh(np.asarray(devices), ("core",))
    in_specs = (PartitionSpec("core"),) * (n_params + len(out_names))
    out_specs = (PartitionSpec("core"),) * len(out_names)
    fn = jax.jit(
        shard_map(_body, mesh=mesh, in_specs=in_specs, out_specs=out_specs,
                  check_rep=False),
        keep_unused=True,
    )
    sharding = NamedSharding(mesh, PartitionSpec("core"))
    extra_dev = [
        jax.device_put(np.zeros((n_cores * s[0], *s[1:]), dt), sharding)
        for s, dt in zero_shapes
    ]

    entry = (fn, in_names, out_names, out_avals, extra_dev, sharding)
    _RUNNERS[key] = entry
    return entry


def run_pjrt(nc, gmaps: dict, n_cores: int):
    fn, in_names, out_names, out_avals, extra_dev, _ = _get_runner(nc, n_cores)
    args = [gmaps[nm] for nm in in_names]
    outs = fn(*args, *extra_dev)
    return {
        nm: np.asarray(o).reshape(n_cores, *out_avals[i].shape)
        for i, (nm, o) in enumerate(zip(out_names, outs))
    }


_CACHE = {}


def run(g: Geom, inputs: dict, reps: int = 1, gmaps_cache=None):
    if gmaps_cache is not None:
        gmaps, NB, TA = gmaps_cache
        key = (g.N, g.M, g.NCORES, NB, TA, reps)
        if key not in _CACHE:
            _CACHE[key] = build_program(g, NB, TA, reps=reps)
        nc = _CACHE[key]
        res = run_pjrt(nc, gmaps, g.NCORES)
    else:
        # streaming path: upload each prepped array while later ones build
        import jax

        state, NB, TA = prep_counts(g, inputs)
        key = (g.N, g.M, g.NCORES, NB, TA, reps)
        if key not in _CACHE:
            _CACHE[key] = build_program(g, NB, TA, reps=reps)
        nc = _CACHE[key]
        fn, in_names, out_names, out_avals, extra_dev, sharding = _get_runner(
            nc, g.NCORES
        )
        dev = {
            nm: jax.device_put(arr, sharding)
            for nm, arr in prep_arrays(g, inputs, state, NB, TA)
        }
        outs = fn(*[dev[nm] for nm in in_names], *extra_dev)
        res = {
            nm: np.asarray(o).reshape(g.NCORES, *out_avals[i].shape)
            for i, (nm, o) in enumerate(zip(out_names, outs))
        }
    outT = res["outT"]  # [NCORES, 128, NPAD] bf16
    out = np.empty((g.N, D), np.float32)
    for c in range(g.NCORES):
        out[c * g.NPC:(c + 1) * g.NPC] = (
            outT[c][:, :g.NPC].astype(np.float32).T
        )
    return out, res


def kernel(**inputs) -> np.ndarray:
    g = Geom()
    out, _ = run(g, inputs)
    return out
